# revision 1
# baseline (speedup 1.0000x reference)
"""3-layer GCN (message passing) + sum-pool + MLP head on 8 Trainium2 cores.

Strategy (all shapes hardcoded; self-contained):
  - Host graph preprocessing: permute nodes into 392 blocks of 128 (49
    blocks/core); nodes split into two gather halves (int16 idx limit) with
    per-block-half edge capacity balanced by a greedy packer.
  - Layer 1 aggregates x*d_inv directly (linearity: A(xW0) = (Ax)W0), so the
    L1 gather table is a replicated input - no dense pre-pass, no AllGather.
  - Tables are bf16 [*, 128]; one-hot routing matrices are exact 0/1 bf16;
    all GCN normalization is folded into d_inv^2 epilogues on the dense
    outputs (commutes with relu since d_inv > 0). Self-loops are a constant
    identity-matmul chunk. L3 folds d_inv[dst] into its one-hot values and
    sum-pools via relu accum_out.
  - Node-sharded dense outputs are AllGather'd between layers; pooled vector
    is AllReduce'd; the tiny MLP head runs replicated on-device.
"""
import sys

import numpy as np

for _p in ("/opt/trn_rl_repo", "/root/.axon_site/_ro/trn_rl_repo"):
    if _p not in sys.path:
        sys.path.append(_p)

import ml_dtypes

import concourse.bacc as bacc
import concourse.bass as bass
import concourse.mybir as mybir
import concourse.tile as tile
from concourse.bass_utils import run_bass_kernel_spmd

# ---------------------------------------------------------------- constants
N = 50000                 # real nodes
P = 128
NB = 392                  # blocks (of 128 node slots)
NP = NB * P               # padded nodes = 50176
NCORES = 8
BPC = NB // NCORES        # 49 blocks per core
ROWS_PC = BPC * P         # 6272 rows per core shard
NHB = NB // 2             # 196 blocks per half
H = NHB * P               # 25088 = gather-half split (int16 idx limit)
CAP = 1152                # edge capacity per block per source-half
NCH = CAP // P            # 9 chunks per half
CHB = 2 * NCH             # 18 edge chunks per block
BATCH_SIZES = [7] * 7                # gather batching of the 49 blocks
IDX_COLS = 2 * (CAP // 16) * BPC     # 7056 idx columns (int16, wrapped by 16)
FW = 128                  # stored table width (bf16)

_CACHED_NC = None
BF16 = ml_dtypes.bfloat16


# ------------------------------------------------------------- host prepro
def _balance_blocks(a_w, b_w, nblocks, cap):
    """Greedy-pack nodes (with per-node loads a_w/b_w) into blocks of <=128
    nodes with per-half loads <= cap. Returns block id per node position."""
    order = np.argsort(-(a_w + b_w), kind="stable")
    la = np.zeros(nblocks, np.int64)
    lb = np.zeros(nblocks, np.int64)
    cnt = np.zeros(nblocks, np.int64)
    out = np.empty(len(a_w), np.int64)
    for i in order:
        na = la + a_w[i]
        nb_ = lb + b_w[i]
        score = np.maximum(na, nb_).astype(np.float64)
        score[(cnt >= P) | (na > cap) | (nb_ > cap)] = np.inf
        j = int(np.argmin(score))
        assert np.isfinite(score[j]), "block packing infeasible; raise CAP"
        out[i] = j
        la[j] = na[j]
        lb[j] = nb_[j]
        cnt[j] += 1
    return out


def _preprocess(x, edge_index):
    src = np.asarray(edge_index[0], np.int64)
    dst = np.asarray(edge_index[1], np.int64)

    deg = np.bincount(dst, minlength=N).astype(np.float64)
    d_inv = 1.0 / np.sqrt(deg + 1.0)

    # ---- split nodes into halves balancing out-edge (as-source) mass
    out_w = np.bincount(src, minlength=N)
    order = np.argsort(-out_w, kind="stable")
    half = np.zeros(N, np.int8)
    tot = [0, 0]
    cnti = [0, 0]
    for i in order:
        h_ = 0 if (tot[0] <= tot[1] and cnti[0] < H) or cnti[1] >= H else 1
        half[i] = h_
        tot[h_] += out_w[i]
        cnti[h_] += 1

    # ---- per-node in-loads split by source half
    sh = half[src]
    a_in = np.bincount(dst[sh == 0], minlength=N)
    b_in = np.bincount(dst[sh == 1], minlength=N)

    # ---- pack each half's nodes into its 196 blocks
    perm_pos = np.empty(N, np.int64)  # node -> permuted id
    for h_ in (0, 1):
        nodes = np.nonzero(half == h_)[0]
        blk = _balance_blocks(a_in[nodes], b_in[nodes], NHB, CAP)
        o2 = np.argsort(blk, kind="stable")
        sb = blk[o2]
        grp_start = np.searchsorted(sb, np.arange(NHB), side="left")
        pos_in_grp = np.arange(len(nodes)) - grp_start[sb]
        perm_pos[nodes[o2]] = (h_ * NHB + sb) * P + pos_in_grp

    # ---- remap edges, group by (dst block, src half)
    psrc = perm_pos[src]
    pdst = perm_pos[dst]
    eb = pdst >> 7              # dst block
    es = pdst & 127             # dst slot
    eh = (psrc >= H).astype(np.int64)
    eidx = psrc - eh * H        # gather idx within half

    key = eb * 2 + eh
    order_e = np.argsort(key, kind="stable")
    key_s = key[order_e]
    cnts = np.bincount(key_s, minlength=NB * 2)
    assert cnts.max() <= CAP, f"block-half overflow: {cnts.max()} > {CAP}"
    starts = np.concatenate([[0], np.cumsum(cnts)[:-1]])
    pos = np.arange(len(key_s)) - starts[key_s]

    # ---- fill per-core device arrays
    idxs = np.zeros((NCORES, 16, IDX_COLS), np.int16)
    dstloc = np.full((NCORES, P, BPC * CHB), 999.0, np.float32)
    w3vals = np.zeros((NCORES, P, BPC * CHB), np.float32)

    g_eb = eb[order_e]
    g_eh = eh[order_e]
    g_core = g_eb // BPC
    g_j = g_eb % BPC                      # core-local block
    bs_arr = np.array(BATCH_SIZES)
    blk2batch = np.repeat(np.arange(len(bs_arr)), bs_arr)
    batch_blk0 = np.concatenate([[0], np.cumsum(bs_arr)[:-1]])
    g_batch = blk2batch[g_j]
    g_k = g_j - batch_blk0[g_batch]       # block within batch

    # dstloc / L3 one-hot values: col = j*18 + h*9 + pos//128, row = pos%128
    col_dw = g_j * CHB + g_eh * NCH + pos // P
    dstloc[g_core, pos % P, col_dw] = es[order_e]
    w3vals[g_core, pos % P, col_dw] = d_inv[dst[order_e]]

    # idx: batch-grouped wrapped layout
    batch_col0 = np.concatenate([[0], np.cumsum(2 * (CAP // 16) * bs_arr)[:-1]])
    call_off = batch_col0[g_batch] + g_eh * (CAP // 16) * bs_arr[g_batch]
    q = g_k * CAP + pos
    idxs[g_core, q % 16, call_off + q // 16] = eidx[order_e]
    idxs_full = np.tile(idxs, (1, 8, 1))  # replicate to 128 partitions

    # ---- bf16 L1 gather table: xg[perm(n), 0:14] = x[n] * d_inv[n]
    xg = np.zeros((NP, FW), BF16)
    xg[perm_pos, :14] = (np.asarray(x, np.float64)
                         * d_inv[:, None]).astype(BF16)
    xg_own = xg.reshape(NCORES, ROWS_PC, FW)

    # ---- per-slot d_inv arrays [core][slot, block]
    dinv1 = np.zeros((NCORES, P, BPC), np.float32)
    dinv2 = np.zeros((NCORES, P, BPC), np.float32)
    blk_all = perm_pos >> 7
    slot_all = perm_pos & 127
    dinv1[blk_all // BPC, slot_all, blk_all % BPC] = d_inv
    dinv2[blk_all // BPC, slot_all, blk_all % BPC] = d_inv * d_inv
    return xg, xg_own, idxs_full, dstloc, w3vals, dinv1, dinv2


# ------------------------------------------------------------ device build
def _build_kernel():
    nc = bacc.Bacc("TRN2", target_bir_lowering=False, debug=False)
    dt = mybir.dt

    xg = nc.dram_tensor("xg", [NP, FW], dt.bfloat16, kind="ExternalInput")
    xgo = nc.dram_tensor("xgo", [ROWS_PC, FW], dt.bfloat16, kind="ExternalInput")
    w0 = nc.dram_tensor("w0", [14, 128], dt.float32, kind="ExternalInput")
    w1 = nc.dram_tensor("w1", [128, 128], dt.float32, kind="ExternalInput")
    w2p = nc.dram_tensor("w2p", [128, FW], dt.float32, kind="ExternalInput")
    fc11w = nc.dram_tensor("fc11w", [32, 16], dt.float32, kind="ExternalInput")
    fc11b = nc.dram_tensor("fc11b", [16, 1], dt.float32, kind="ExternalInput")
    fc12w = nc.dram_tensor("fc12w", [16, 1], dt.float32, kind="ExternalInput")
    fc12b = nc.dram_tensor("fc12b", [1, 1], dt.float32, kind="ExternalInput")
    iota = nc.dram_tensor("iota", [P, P], dt.float32, kind="ExternalInput")
    iotab = nc.dram_tensor("iotab", [P, CHB * P], dt.float32, kind="ExternalInput")
    ident = nc.dram_tensor("ident", [P, P], dt.bfloat16, kind="ExternalInput")
    prange = nc.dram_tensor("prange", [P, 1], dt.float32, kind="ExternalInput")
    dinv1 = nc.dram_tensor("dinv1", [P, BPC], dt.float32, kind="ExternalInput")
    dinv2 = nc.dram_tensor("dinv2", [P, BPC], dt.float32, kind="ExternalInput")
    idxs = nc.dram_tensor("idxs", [P, IDX_COLS], dt.int16, kind="ExternalInput")
    dstloc = nc.dram_tensor("dstloc", [P, BPC * CHB], dt.float32, kind="ExternalInput")
    w3v = nc.dram_tensor("w3v", [P, BPC * CHB], dt.float32, kind="ExternalInput")
    out = nc.dram_tensor("out", [1, 1], dt.float32, kind="ExternalOutput")

    bs_arr = np.array(BATCH_SIZES)
    batch_col0 = np.concatenate([[0], np.cumsum(2 * (CAP // 16) * bs_arr)[:-1]])
    batch_blk0 = np.concatenate([[0], np.cumsum(bs_arr)[:-1]])

    with tile.TileContext(nc) as tc:
        with (
            tc.tile_pool(name="const", bufs=1) as cst,
            tc.tile_pool(name="ga", bufs=2) as gap,
            tc.tile_pool(name="gb", bufs=2) as gbp,
            tc.tile_pool(name="oh", bufs=12) as ohp,
            tc.tile_pool(name="eq", bufs=2) as eqp,
            tc.tile_pool(name="rl", bufs=3) as rlp,
            tc.tile_pool(name="st", bufs=2) as stp,
            tc.tile_pool(name="misc", bufs=1) as msc,
            tc.tile_pool(name="psA", bufs=2, space="PSUM") as psa,
            tc.tile_pool(name="psX", bufs=2, space="PSUM") as psx,
            tc.tile_pool(name="psD", bufs=2, space="PSUM") as psd,
            tc.tile_pool(name="psP", bufs=1, space="PSUM") as psp,
            tc.tile_pool(name="dram", bufs=1, space="DRAM") as drm,
        ):
            # resident constants
            idxs_t = cst.tile([P, IDX_COLS], dt.int16)
            dstloc_t = cst.tile([P, BPC * CHB], dt.float32)
            w3v_t = cst.tile([P, BPC * CHB], dt.float32)
            iota_t = cst.tile([P, P], dt.float32)
            iotab_t = cst.tile([P, CHB * P], dt.float32)
            ident_t = cst.tile([P, P], dt.bfloat16)
            prange_t = cst.tile([P, 1], dt.float32)
            dinv1_t = cst.tile([P, BPC], dt.float32)
            dinv2_t = cst.tile([P, BPC], dt.float32)
            w0_t = cst.tile([14, 128], dt.float32)
            w1_t = cst.tile([128, 128], dt.float32)
            w2p_t = cst.tile([128, FW], dt.float32)
            fc11w_t = cst.tile([32, 16], dt.float32)
            fc11b_t = cst.tile([16, 1], dt.float32)
            fc12w_t = cst.tile([16, 1], dt.float32)
            fc12b_t = cst.tile([1, 1], dt.float32)
            for t_, d_ in (
                (idxs_t, idxs), (dstloc_t, dstloc), (w3v_t, w3v),
                (iota_t, iota), (iotab_t, iotab),
                (ident_t, ident), (prange_t, prange),
                (dinv1_t, dinv1), (dinv2_t, dinv2),
                (w0_t, w0), (w1_t, w1), (w2p_t, w2p),
                (fc11w_t, fc11w), (fc11b_t, fc11b), (fc12w_t, fc12w),
                (fc12b_t, fc12b),
            ):
                nc.sync.dma_start(t_[:], d_[:])

            # internal DRAM (bf16 tables)
            g2s_t = drm.tile([ROWS_PC, FW], dt.bfloat16)
            g2_t = drm.tile([NP, FW], dt.bfloat16)
            g3s_t = drm.tile([ROWS_PC, FW], dt.bfloat16)
            g3_t = drm.tile([NP, FW], dt.bfloat16)
            pool_in = drm.tile([32, 1], dt.float32)
            pool_out = drm.tile([32, 1], dt.float32)

            pooled_cols = msc.tile([32, BPC], dt.float32)

            def layer(lnum, h_src, h_self, h_shard):
                for t, bs in enumerate(BATCH_SIZES):
                    ic0 = int(batch_col0[t])
                    b0 = int(batch_blk0[t])
                    ga = gap.tile([P, NCH * bs, FW], dt.bfloat16, tag="ga")
                    gb = gbp.tile([P, NCH * bs, FW], dt.bfloat16, tag="gb")
                    nc.gpsimd.dma_gather(
                        ga[:], h_src[0:H, :],
                        idxs_t[:, ic0 : ic0 + (CAP // 16) * bs],
                        CAP * bs, CAP * bs, FW, single_packet=False)
                    nc.gpsimd.dma_gather(
                        gb[:], h_src[H:NP, :],
                        idxs_t[:, ic0 + (CAP // 16) * bs : ic0 + 2 * (CAP // 16) * bs],
                        CAP * bs, CAP * bs, FW, single_packet=False)
                    gs = gap.tile([P, bs, FW], dt.bfloat16, tag="gs")
                    nc.sync.dma_start(
                        gs[:],
                        h_self[b0 * P : (b0 + bs) * P, :].rearrange(
                            "(g p) f -> p g f", p=P))
                    if lnum != 3:
                        dstage = stp.tile([P, bs, FW], dt.bfloat16, tag="dnst")
                    for k in range(bs):
                        j = b0 + k
                        agg = psa.tile([P, P], dt.float32, tag="agg")
                        # build all 18 one-hot chunks of this block at once
                        iview = iotab_t[:].rearrange("p (c j) -> p c j", j=P)
                        dview = dstloc_t[:, j * CHB : (j + 1) * CHB].rearrange(
                            "p (c o) -> p c o", o=1).to_broadcast([P, CHB, P])
                        ohb = ohp.tile([P, CHB, P], dt.bfloat16, tag="ohb")
                        if lnum == 3:
                            eqt = eqp.tile([P, CHB, P], dt.float32, tag="eqt")
                            nc.vector.tensor_tensor(
                                eqt[:], iview, dview, mybir.AluOpType.is_equal)
                            wview = w3v_t[:, j * CHB : (j + 1) * CHB].rearrange(
                                "p (c o) -> p c o", o=1).to_broadcast(
                                    [P, CHB, P])
                            nc.vector.tensor_tensor(
                                ohb[:], eqt[:], wview, mybir.AluOpType.mult)
                        else:
                            nc.vector.tensor_tensor(
                                ohb[:], iview, dview, mybir.AluOpType.is_equal)
                        for c in range(CHB):
                            g = ga if c < NCH else gb
                            cc = c if c < NCH else c - NCH
                            nc.tensor.matmul(
                                agg[:], g[:, NCH * k + cc, :], ohb[:, c, :],
                                start=(c == 0), stop=False)
                        # self-loop chunk: identity (L1/L2) or d_inv diagonal
                        if lnum == 3:
                            ohs = ohp.tile([P, P], dt.bfloat16, tag="ohs")
                            nc.vector.tensor_scalar(
                                ohs[:], iota_t[:],
                                prange_t[:, 0:1], dinv1_t[:, j : j + 1],
                                mybir.AluOpType.is_equal, mybir.AluOpType.mult)
                            nc.tensor.matmul(
                                agg[:], gs[:, k, :], ohs[:],
                                start=False, stop=True)
                        else:
                            nc.tensor.matmul(
                                agg[:], gs[:, k, :], ident_t[:],
                                start=False, stop=True)

                        if lnum == 1:
                            # ZT = W0^T @ agg_x[0:14]; relu; dense W1; epilogue
                            axs = rlp.tile([14, P], dt.float32, tag="axs")
                            nc.vector.tensor_copy(axs[:], agg[0:14, :])
                            zt = psx.tile([P, P], dt.float32, tag="zt")
                            nc.tensor.matmul(
                                zt[:], w0_t[:], axs[:], start=True, stop=True)
                            rT = rlp.tile([P, P], dt.float32, tag="rT")
                            nc.scalar.activation(
                                rT[:], zt[:],
                                mybir.ActivationFunctionType.Relu)
                            h_ps = psd.tile([P, FW], dt.float32, tag="dnps")
                            nc.tensor.matmul(
                                h_ps[:], rT[:], w1_t[:], start=True, stop=True)
                            nc.vector.tensor_scalar(
                                dstage[:, k, :], h_ps[:],
                                dinv2_t[:, j : j + 1], None,
                                mybir.AluOpType.mult)
                        elif lnum == 2:
                            rT = rlp.tile([P, P], dt.float32, tag="rT")
                            nc.scalar.activation(
                                rT[:], agg[:],
                                mybir.ActivationFunctionType.Relu)
                            h_ps = psd.tile([P, FW], dt.float32, tag="dnps")
                            nc.tensor.matmul(
                                h_ps[:], rT[:], w2p_t[:], start=True, stop=True)
                            nc.vector.tensor_scalar(
                                dstage[:, k, :], h_ps[:],
                                dinv2_t[:, j : j + 1], None,
                                mybir.AluOpType.mult)
                        else:
                            # L3: out3T = relu(agg[0:32]); pooled col = row sum
                            r3 = rlp.tile([32, P], dt.float32, tag="r3")
                            nc.scalar.activation(
                                r3[:], agg[0:32, :],
                                mybir.ActivationFunctionType.Relu,
                                accum_out=pooled_cols[:, j : j + 1])
                    if lnum != 3:
                        r0 = b0 * P
                        nc.sync.dma_start(
                            h_shard[r0 : r0 + bs * P, :].rearrange(
                                "(g p) f -> p g f", p=P),
                            dstage[:, :bs, :])

            # L1 (x-aggregation)
            layer(1, xg, xgo, g2s_t)
            nc.gpsimd.collective_compute(
                "AllGather", mybir.AluOpType.bypass,
                replica_groups=[list(range(NCORES))],
                ins=[g2s_t.opt()], outs=[g2_t.opt()])
            # L2
            layer(2, g2_t, g2s_t, g3s_t)
            nc.gpsimd.collective_compute(
                "AllGather", mybir.AluOpType.bypass,
                replica_groups=[list(range(NCORES))],
                ins=[g3s_t.opt()], outs=[g3_t.opt()])
            # L3 + pooling
            layer(3, g3_t, g3s_t, None)
            pooled = msc.tile([32, 1], dt.float32)
            nc.vector.tensor_reduce(
                pooled[:], pooled_cols[:],
                axis=mybir.AxisListType.X, op=mybir.AluOpType.add)

            # global pool AllReduce + MLP head (replicated)
            nc.sync.dma_start(pool_in[:], pooled[:])
            nc.gpsimd.collective_compute(
                "AllReduce", mybir.AluOpType.add,
                replica_groups=[list(range(NCORES))],
                ins=[pool_in.opt()], outs=[pool_out.opt()])
            pooled_g = msc.tile([32, 1], dt.float32)
            nc.sync.dma_start(pooled_g[:], pool_out[:])
            ps16 = psp.tile([16, 1], dt.float32, tag="mlp")
            nc.tensor.matmul(ps16[:], fc11w_t[:], pooled_g[:], start=True, stop=True)
            a16 = msc.tile([16, 1], dt.float32)
            nc.scalar.activation(
                a16[:], ps16[:], mybir.ActivationFunctionType.Relu,
                bias=fc11b_t[:])
            ps1 = psp.tile([1, 1], dt.float32, tag="mlp")
            nc.tensor.matmul(ps1[:], fc12w_t[:], a16[:], start=True, stop=True)
            o1 = msc.tile([1, 1], dt.float32)
            nc.scalar.activation(
                o1[:], ps1[:], mybir.ActivationFunctionType.Identity,
                bias=fc12b_t[:])
            nc.sync.dma_start(out[:], o1[:])

    nc.compile()
    return nc


def _get_nc():
    global _CACHED_NC
    if _CACHED_NC is None:
        _CACHED_NC = _build_kernel()
    return _CACHED_NC


def _make_in_maps(inputs):
    x = np.asarray(inputs["x"], np.float32)
    edge_index = np.asarray(inputs["edge_index"])
    xg, xg_own, idxs, dstloc, w3vals, dinv1, dinv2 = _preprocess(x, edge_index)

    w2p = np.zeros((128, FW), np.float32)
    w2p[:, :32] = np.asarray(inputs["W2"], np.float32)
    common = {
        "xg": xg,
        "w0": np.asarray(inputs["W0"], np.float32),
        "w1": np.asarray(inputs["W1"], np.float32),
        "w2p": w2p,
        "fc11w": np.asarray(inputs["fc11_w"], np.float32),
        "fc11b": np.asarray(inputs["fc11_b"], np.float32).reshape(16, 1),
        "fc12w": np.asarray(inputs["fc12_w"], np.float32),
        "fc12b": np.asarray(inputs["fc12_b"], np.float32).reshape(1, 1),
        "iota": np.tile(np.arange(P, dtype=np.float32), (P, 1)),
        "iotab": np.tile(np.arange(P, dtype=np.float32), (P, CHB)),
        "ident": np.eye(P, dtype=BF16),
        "prange": np.arange(P, dtype=np.float32).reshape(P, 1),
    }
    return [
        {**common, "xgo": np.ascontiguousarray(xg_own[c]), "idxs": idxs[c],
         "dstloc": dstloc[c], "w3v": w3vals[c],
         "dinv1": dinv1[c], "dinv2": dinv2[c]}
        for c in range(NCORES)
    ]


def run(trace=False, _inputs=None, **inputs):
    if _inputs is not None:
        inputs = _inputs
    in_maps = _make_in_maps(inputs)
    nc = _get_nc()
    res = run_bass_kernel_spmd(
        nc, in_maps, core_ids=list(range(NCORES)), trace=trace)
    y = np.asarray(res.results[0]["out"], np.float32).reshape(1)
    return y, res


def kernel(**inputs) -> np.ndarray:
    y, _ = run(**inputs)
    return y



# revision 3
# speedup vs baseline: 1.4948x; 1.4948x over previous
"""3-layer GCN (message passing) + sum-pool + MLP head on 8 Trainium2 cores.

Strategy (all shapes hardcoded; self-contained):
  - Host graph preprocessing: permute nodes into 392 blocks of 128 (49
    blocks/core); nodes split into two gather halves (int16 idx limit) with
    per-block-half edge capacity balanced by a greedy packer.
  - Layer 1 aggregates x*d_inv directly (linearity: A(xW0) = (Ax)W0), so the
    L1 gather table is a replicated input - no dense pre-pass, no AllGather.
  - Tables are bf16 [*, 128]; one-hot routing matrices are exact 0/1 bf16;
    all GCN normalization is folded into d_inv^2 epilogues on the dense
    outputs (commutes with relu since d_inv > 0). Self-loops are a constant
    identity-matmul chunk. L3 folds d_inv[dst] into its one-hot values and
    sum-pools via relu accum_out.
  - Node-sharded dense outputs are AllGather'd between layers; pooled vector
    is AllReduce'd; the tiny MLP head runs replicated on-device.
"""
import sys

import numpy as np

for _p in ("/opt/trn_rl_repo", "/root/.axon_site/_ro/trn_rl_repo"):
    if _p not in sys.path:
        sys.path.append(_p)

import ml_dtypes

import concourse.bacc as bacc
import concourse.bass as bass
import concourse.mybir as mybir
import concourse.tile as tile
from concourse.bass_utils import run_bass_kernel_spmd

# ---------------------------------------------------------------- constants
N = 50000                 # real nodes
P = 128
NB = 392                  # blocks (of 128 node slots)
NP = NB * P               # padded nodes = 50176
NCORES = 8
BPC = NB // NCORES        # 49 blocks per core
ROWS_PC = BPC * P         # 6272 rows per core shard
NHB = NB // 2             # 196 blocks per half
H = NHB * P               # 25088 = gather-half split (int16 idx limit)
CAP = 1152                # edge capacity per block per source-half
NCH = CAP // P            # 9 chunks per half
CHB = 2 * NCH             # 18 edge chunks per block
BATCH_SIZES = [7] * 7                # gather batching of the 49 blocks
IDX_COLS = 2 * (CAP // 16) * BPC     # 7056 idx columns (int16, wrapped by 16)
FW = 128                  # stored table width (bf16)

_CACHED_NC = None
BF16 = ml_dtypes.bfloat16


# ------------------------------------------------------------- host prepro
def _balance_blocks(a_w, b_w, nblocks, cap):
    """Greedy-pack nodes (with per-node loads a_w/b_w) into blocks of <=128
    nodes with per-half loads <= cap. Returns block id per node position."""
    order = np.argsort(-(a_w + b_w), kind="stable")
    la = np.zeros(nblocks, np.int64)
    lb = np.zeros(nblocks, np.int64)
    cnt = np.zeros(nblocks, np.int64)
    out = np.empty(len(a_w), np.int64)
    for i in order:
        na = la + a_w[i]
        nb_ = lb + b_w[i]
        score = np.maximum(na, nb_).astype(np.float64)
        score[(cnt >= P) | (na > cap) | (nb_ > cap)] = np.inf
        j = int(np.argmin(score))
        assert np.isfinite(score[j]), "block packing infeasible; raise CAP"
        out[i] = j
        la[j] = na[j]
        lb[j] = nb_[j]
        cnt[j] += 1
    return out


def _preprocess(x, edge_index):
    src = np.asarray(edge_index[0], np.int64)
    dst = np.asarray(edge_index[1], np.int64)

    deg = np.bincount(dst, minlength=N).astype(np.float64)
    d_inv = 1.0 / np.sqrt(deg + 1.0)

    # ---- split nodes into halves balancing out-edge (as-source) mass
    out_w = np.bincount(src, minlength=N)
    order = np.argsort(-out_w, kind="stable")
    half = np.zeros(N, np.int8)
    tot = [0, 0]
    cnti = [0, 0]
    for i in order:
        h_ = 0 if (tot[0] <= tot[1] and cnti[0] < H) or cnti[1] >= H else 1
        half[i] = h_
        tot[h_] += out_w[i]
        cnti[h_] += 1

    # ---- per-node in-loads split by source half
    sh = half[src]
    a_in = np.bincount(dst[sh == 0], minlength=N)
    b_in = np.bincount(dst[sh == 1], minlength=N)

    # ---- pack each half's nodes into its 196 blocks
    perm_pos = np.empty(N, np.int64)  # node -> permuted id
    for h_ in (0, 1):
        nodes = np.nonzero(half == h_)[0]
        blk = _balance_blocks(a_in[nodes], b_in[nodes], NHB, CAP)
        o2 = np.argsort(blk, kind="stable")
        sb = blk[o2]
        grp_start = np.searchsorted(sb, np.arange(NHB), side="left")
        pos_in_grp = np.arange(len(nodes)) - grp_start[sb]
        perm_pos[nodes[o2]] = (h_ * NHB + sb) * P + pos_in_grp

    # ---- remap edges, group by (dst block, src half)
    psrc = perm_pos[src]
    pdst = perm_pos[dst]
    eb = pdst >> 7              # dst block
    es = pdst & 127             # dst slot
    eh = (psrc >= H).astype(np.int64)
    eidx = psrc - eh * H        # gather idx within half

    key = eb * 2 + eh
    order_e = np.argsort(key, kind="stable")
    key_s = key[order_e]
    cnts = np.bincount(key_s, minlength=NB * 2)
    assert cnts.max() <= CAP, f"block-half overflow: {cnts.max()} > {CAP}"
    starts = np.concatenate([[0], np.cumsum(cnts)[:-1]])
    pos = np.arange(len(key_s)) - starts[key_s]

    # ---- fill per-core device arrays
    idxs = np.zeros((NCORES, 16, IDX_COLS), np.int16)
    dstloc = np.full((NCORES, P, BPC * CHB), 999.0, np.float32)
    w3vals = np.zeros((NCORES, P, BPC * CHB), np.float32)

    g_eb = eb[order_e]
    g_eh = eh[order_e]
    g_core = g_eb // BPC
    g_j = g_eb % BPC                      # core-local block
    bs_arr = np.array(BATCH_SIZES)
    blk2batch = np.repeat(np.arange(len(bs_arr)), bs_arr)
    batch_blk0 = np.concatenate([[0], np.cumsum(bs_arr)[:-1]])
    g_batch = blk2batch[g_j]
    g_k = g_j - batch_blk0[g_batch]       # block within batch

    # dstloc / L3 one-hot values: col = j*18 + h*9 + pos//128, row = pos%128
    col_dw = g_j * CHB + g_eh * NCH + pos // P
    dstloc[g_core, pos % P, col_dw] = es[order_e]
    w3vals[g_core, pos % P, col_dw] = d_inv[dst[order_e]]

    # idx: batch-grouped wrapped layout
    batch_col0 = np.concatenate([[0], np.cumsum(2 * (CAP // 16) * bs_arr)[:-1]])
    call_off = batch_col0[g_batch] + g_eh * (CAP // 16) * bs_arr[g_batch]
    q = g_k * CAP + pos
    idxs[g_core, q % 16, call_off + q // 16] = eidx[order_e]
    idxs_full = np.tile(idxs, (1, 8, 1))  # replicate to 128 partitions

    # ---- bf16 L1 gather table: xg[perm(n), 0:14] = x[n] * d_inv[n]
    xg = np.zeros((NP, FW), BF16)
    xg[perm_pos, :14] = (np.asarray(x, np.float64)
                         * d_inv[:, None]).astype(BF16)
    xg_own = xg.reshape(NCORES, ROWS_PC, FW)

    # ---- per-slot d_inv arrays [core][slot, block]
    dinv1 = np.zeros((NCORES, P, BPC), np.float32)
    dinv2 = np.zeros((NCORES, P, BPC), np.float32)
    blk_all = perm_pos >> 7
    slot_all = perm_pos & 127
    dinv1[blk_all // BPC, slot_all, blk_all % BPC] = d_inv
    dinv2[blk_all // BPC, slot_all, blk_all % BPC] = d_inv * d_inv
    return xg, xg_own, idxs_full, dstloc, w3vals, dinv1, dinv2


# ------------------------------------------------------------ device build
def _build_kernel():
    nc = bacc.Bacc("TRN2", target_bir_lowering=False, debug=False,
                   num_swdge_queues=4)
    dt = mybir.dt

    xg = nc.dram_tensor("xg", [NP, FW], dt.bfloat16, kind="ExternalInput")
    xgo = nc.dram_tensor("xgo", [ROWS_PC, FW], dt.bfloat16, kind="ExternalInput")
    w0 = nc.dram_tensor("w0", [14, 128], dt.float32, kind="ExternalInput")
    w1 = nc.dram_tensor("w1", [128, 128], dt.float32, kind="ExternalInput")
    w2p = nc.dram_tensor("w2p", [128, FW], dt.float32, kind="ExternalInput")
    fc11w = nc.dram_tensor("fc11w", [32, 16], dt.float32, kind="ExternalInput")
    fc11b = nc.dram_tensor("fc11b", [16, 1], dt.float32, kind="ExternalInput")
    fc12w = nc.dram_tensor("fc12w", [16, 1], dt.float32, kind="ExternalInput")
    fc12b = nc.dram_tensor("fc12b", [1, 1], dt.float32, kind="ExternalInput")
    iota = nc.dram_tensor("iota", [P, P], dt.float32, kind="ExternalInput")
    iotab = nc.dram_tensor("iotab", [P, CHB * P], dt.float32, kind="ExternalInput")
    ident = nc.dram_tensor("ident", [P, P], dt.bfloat16, kind="ExternalInput")
    prange = nc.dram_tensor("prange", [P, 1], dt.float32, kind="ExternalInput")
    dinv1 = nc.dram_tensor("dinv1", [P, BPC], dt.float32, kind="ExternalInput")
    dinv2 = nc.dram_tensor("dinv2", [P, BPC], dt.float32, kind="ExternalInput")
    idxs = nc.dram_tensor("idxs", [P, IDX_COLS], dt.int16, kind="ExternalInput")
    dstloc = nc.dram_tensor("dstloc", [P, BPC * CHB], dt.float32, kind="ExternalInput")
    w3v = nc.dram_tensor("w3v", [P, BPC * CHB], dt.float32, kind="ExternalInput")
    out = nc.dram_tensor("out", [1, 1], dt.float32, kind="ExternalOutput")

    bs_arr = np.array(BATCH_SIZES)
    batch_col0 = np.concatenate([[0], np.cumsum(2 * (CAP // 16) * bs_arr)[:-1]])
    batch_blk0 = np.concatenate([[0], np.cumsum(bs_arr)[:-1]])

    with tile.TileContext(nc) as tc:
        with (
            tc.tile_pool(name="const", bufs=1) as cst,
            tc.tile_pool(name="ga", bufs=2) as gap,
            tc.tile_pool(name="gb", bufs=2) as gbp,
            tc.tile_pool(name="oh", bufs=12) as ohp,
            tc.tile_pool(name="eq", bufs=2) as eqp,
            tc.tile_pool(name="rl", bufs=3) as rlp,
            tc.tile_pool(name="st", bufs=2) as stp,
            tc.tile_pool(name="misc", bufs=1) as msc,
            tc.tile_pool(name="psA", bufs=2, space="PSUM") as psa,
            tc.tile_pool(name="psX", bufs=2, space="PSUM") as psx,
            tc.tile_pool(name="psD", bufs=2, space="PSUM") as psd,
            tc.tile_pool(name="psP", bufs=1, space="PSUM") as psp,
            tc.tile_pool(name="dram", bufs=1, space="DRAM") as drm,
        ):
            # resident constants
            idxs_t = cst.tile([P, IDX_COLS], dt.int16)
            dstloc_t = cst.tile([P, BPC * CHB], dt.float32)
            w3v_t = cst.tile([P, BPC * CHB], dt.float32)
            iota_t = cst.tile([P, P], dt.float32)
            iotab_t = cst.tile([P, CHB * P], dt.float32)
            ident_t = cst.tile([P, P], dt.bfloat16)
            prange_t = cst.tile([P, 1], dt.float32)
            dinv1_t = cst.tile([P, BPC], dt.float32)
            dinv2_t = cst.tile([P, BPC], dt.float32)
            w0_t = cst.tile([14, 128], dt.float32)
            w1_t = cst.tile([128, 128], dt.float32)
            w2p_t = cst.tile([128, FW], dt.float32)
            fc11w_t = cst.tile([32, 16], dt.float32)
            fc11b_t = cst.tile([16, 1], dt.float32)
            fc12w_t = cst.tile([16, 1], dt.float32)
            fc12b_t = cst.tile([1, 1], dt.float32)
            for t_, d_ in (
                (idxs_t, idxs), (dstloc_t, dstloc), (w3v_t, w3v),
                (iota_t, iota), (iotab_t, iotab),
                (ident_t, ident), (prange_t, prange),
                (dinv1_t, dinv1), (dinv2_t, dinv2),
                (w0_t, w0), (w1_t, w1), (w2p_t, w2p),
                (fc11w_t, fc11w), (fc11b_t, fc11b), (fc12w_t, fc12w),
                (fc12b_t, fc12b),
            ):
                nc.sync.dma_start(t_[:], d_[:])

            # internal DRAM (bf16 tables)
            g2s_t = drm.tile([ROWS_PC, FW], dt.bfloat16)
            g2_t = drm.tile([NP, FW], dt.bfloat16)
            g3s_t = drm.tile([ROWS_PC, FW], dt.bfloat16)
            g3_t = drm.tile([NP, FW], dt.bfloat16)
            pool_in = drm.tile([32, 1], dt.float32)
            pool_out = drm.tile([32, 1], dt.float32)

            pooled_cols = msc.tile([32, BPC], dt.float32)

            def layer(lnum, h_src, h_self, h_shard):
                for t, bs in enumerate(BATCH_SIZES):
                    ic0 = int(batch_col0[t])
                    b0 = int(batch_blk0[t])
                    ga = gap.tile([P, NCH * bs, FW], dt.bfloat16, tag="ga")
                    gb = gbp.tile([P, NCH * bs, FW], dt.bfloat16, tag="gb")
                    nc.gpsimd.dma_gather(
                        ga[:], h_src[0:H, :],
                        idxs_t[:, ic0 : ic0 + (CAP // 16) * bs],
                        CAP * bs, CAP * bs, FW, single_packet=False,
                        queue_num=(2 * t) % 4)
                    nc.gpsimd.dma_gather(
                        gb[:], h_src[H:NP, :],
                        idxs_t[:, ic0 + (CAP // 16) * bs : ic0 + 2 * (CAP // 16) * bs],
                        CAP * bs, CAP * bs, FW, single_packet=False,
                        queue_num=(2 * t + 1) % 4)
                    gs = gap.tile([P, bs, FW], dt.bfloat16, tag="gs")
                    nc.sync.dma_start(
                        gs[:],
                        h_self[b0 * P : (b0 + bs) * P, :].rearrange(
                            "(g p) f -> p g f", p=P))
                    if lnum != 3:
                        dstage = stp.tile([P, bs, FW], dt.bfloat16, tag="dnst")
                    for k in range(bs):
                        j = b0 + k
                        agg = psa.tile([P, P], dt.float32, tag="agg")
                        # build all 18 one-hot chunks of this block at once
                        iview = iotab_t[:].rearrange("p (c j) -> p c j", j=P)
                        dview = dstloc_t[:, j * CHB : (j + 1) * CHB].rearrange(
                            "p (c o) -> p c o", o=1).to_broadcast([P, CHB, P])
                        ohb = ohp.tile([P, CHB, P], dt.bfloat16, tag="ohb")
                        if lnum == 3:
                            eqt = eqp.tile([P, CHB, P], dt.float32, tag="eqt")
                            nc.vector.tensor_tensor(
                                eqt[:], iview, dview, mybir.AluOpType.is_equal)
                            wview = w3v_t[:, j * CHB : (j + 1) * CHB].rearrange(
                                "p (c o) -> p c o", o=1).to_broadcast(
                                    [P, CHB, P])
                            nc.vector.tensor_tensor(
                                ohb[:], eqt[:], wview, mybir.AluOpType.mult)
                        else:
                            nc.vector.tensor_tensor(
                                ohb[:], iview, dview, mybir.AluOpType.is_equal)
                        for c in range(CHB):
                            g = ga if c < NCH else gb
                            cc = c if c < NCH else c - NCH
                            nc.tensor.matmul(
                                agg[:], g[:, NCH * k + cc, :], ohb[:, c, :],
                                start=(c == 0), stop=False)
                        # self-loop chunk: identity (L1/L2) or d_inv diagonal
                        if lnum == 3:
                            ohs = ohp.tile([P, P], dt.bfloat16, tag="ohs")
                            nc.vector.tensor_scalar(
                                ohs[:], iota_t[:],
                                prange_t[:, 0:1], dinv1_t[:, j : j + 1],
                                mybir.AluOpType.is_equal, mybir.AluOpType.mult)
                            nc.tensor.matmul(
                                agg[:], gs[:, k, :], ohs[:],
                                start=False, stop=True)
                        else:
                            nc.tensor.matmul(
                                agg[:], gs[:, k, :], ident_t[:],
                                start=False, stop=True)

                        if lnum == 1:
                            # ZT = W0^T @ agg_x[0:14]; relu; dense W1; epilogue
                            axs = rlp.tile([14, P], dt.float32, tag="axs")
                            nc.vector.tensor_copy(axs[:], agg[0:14, :])
                            zt = psx.tile([P, P], dt.float32, tag="zt")
                            nc.tensor.matmul(
                                zt[:], w0_t[:], axs[:], start=True, stop=True)
                            rT = rlp.tile([P, P], dt.float32, tag="rT")
                            nc.scalar.activation(
                                rT[:], zt[:],
                                mybir.ActivationFunctionType.Relu)
                            h_ps = psd.tile([P, FW], dt.float32, tag="dnps")
                            nc.tensor.matmul(
                                h_ps[:], rT[:], w1_t[:], start=True, stop=True)
                            nc.vector.tensor_scalar(
                                dstage[:, k, :], h_ps[:],
                                dinv2_t[:, j : j + 1], None,
                                mybir.AluOpType.mult)
                        elif lnum == 2:
                            rT = rlp.tile([P, P], dt.float32, tag="rT")
                            nc.scalar.activation(
                                rT[:], agg[:],
                                mybir.ActivationFunctionType.Relu)
                            h_ps = psd.tile([P, FW], dt.float32, tag="dnps")
                            nc.tensor.matmul(
                                h_ps[:], rT[:], w2p_t[:], start=True, stop=True)
                            nc.vector.tensor_scalar(
                                dstage[:, k, :], h_ps[:],
                                dinv2_t[:, j : j + 1], None,
                                mybir.AluOpType.mult)
                        else:
                            # L3: out3T = relu(agg[0:32]); pooled col = row sum
                            r3 = rlp.tile([32, P], dt.float32, tag="r3")
                            nc.scalar.activation(
                                r3[:], agg[0:32, :],
                                mybir.ActivationFunctionType.Relu,
                                accum_out=pooled_cols[:, j : j + 1])
                    if lnum != 3:
                        r0 = b0 * P
                        nc.sync.dma_start(
                            h_shard[r0 : r0 + bs * P, :].rearrange(
                                "(g p) f -> p g f", p=P),
                            dstage[:, :bs, :])

            # L1 (x-aggregation)
            layer(1, xg, xgo, g2s_t)
            nc.gpsimd.collective_compute(
                "AllGather", mybir.AluOpType.bypass,
                replica_groups=[list(range(NCORES))],
                ins=[g2s_t.opt()], outs=[g2_t.opt()])
            # L2
            layer(2, g2_t, g2s_t, g3s_t)
            nc.gpsimd.collective_compute(
                "AllGather", mybir.AluOpType.bypass,
                replica_groups=[list(range(NCORES))],
                ins=[g3s_t.opt()], outs=[g3_t.opt()])
            # L3 + pooling
            layer(3, g3_t, g3s_t, None)
            pooled = msc.tile([32, 1], dt.float32)
            nc.vector.tensor_reduce(
                pooled[:], pooled_cols[:],
                axis=mybir.AxisListType.X, op=mybir.AluOpType.add)

            # global pool AllReduce + MLP head (replicated)
            nc.sync.dma_start(pool_in[:], pooled[:])
            nc.gpsimd.collective_compute(
                "AllReduce", mybir.AluOpType.add,
                replica_groups=[list(range(NCORES))],
                ins=[pool_in.opt()], outs=[pool_out.opt()])
            pooled_g = msc.tile([32, 1], dt.float32)
            nc.sync.dma_start(pooled_g[:], pool_out[:])
            ps16 = psp.tile([16, 1], dt.float32, tag="mlp")
            nc.tensor.matmul(ps16[:], fc11w_t[:], pooled_g[:], start=True, stop=True)
            a16 = msc.tile([16, 1], dt.float32)
            nc.scalar.activation(
                a16[:], ps16[:], mybir.ActivationFunctionType.Relu,
                bias=fc11b_t[:])
            ps1 = psp.tile([1, 1], dt.float32, tag="mlp")
            nc.tensor.matmul(ps1[:], fc12w_t[:], a16[:], start=True, stop=True)
            o1 = msc.tile([1, 1], dt.float32)
            nc.scalar.activation(
                o1[:], ps1[:], mybir.ActivationFunctionType.Identity,
                bias=fc12b_t[:])
            nc.sync.dma_start(out[:], o1[:])

    nc.compile()
    return nc


def _get_nc():
    global _CACHED_NC
    if _CACHED_NC is None:
        _CACHED_NC = _build_kernel()
    return _CACHED_NC


def _make_in_maps(inputs):
    x = np.asarray(inputs["x"], np.float32)
    edge_index = np.asarray(inputs["edge_index"])
    xg, xg_own, idxs, dstloc, w3vals, dinv1, dinv2 = _preprocess(x, edge_index)

    w2p = np.zeros((128, FW), np.float32)
    w2p[:, :32] = np.asarray(inputs["W2"], np.float32)
    common = {
        "xg": xg,
        "w0": np.asarray(inputs["W0"], np.float32),
        "w1": np.asarray(inputs["W1"], np.float32),
        "w2p": w2p,
        "fc11w": np.asarray(inputs["fc11_w"], np.float32),
        "fc11b": np.asarray(inputs["fc11_b"], np.float32).reshape(16, 1),
        "fc12w": np.asarray(inputs["fc12_w"], np.float32),
        "fc12b": np.asarray(inputs["fc12_b"], np.float32).reshape(1, 1),
        "iota": np.tile(np.arange(P, dtype=np.float32), (P, 1)),
        "iotab": np.tile(np.arange(P, dtype=np.float32), (P, CHB)),
        "ident": np.eye(P, dtype=BF16),
        "prange": np.arange(P, dtype=np.float32).reshape(P, 1),
    }
    return [
        {**common, "xgo": np.ascontiguousarray(xg_own[c]), "idxs": idxs[c],
         "dstloc": dstloc[c], "w3v": w3vals[c],
         "dinv1": dinv1[c], "dinv2": dinv2[c]}
        for c in range(NCORES)
    ]


def run(trace=False, _inputs=None, **inputs):
    if _inputs is not None:
        inputs = _inputs
    in_maps = _make_in_maps(inputs)
    nc = _get_nc()
    res = run_bass_kernel_spmd(
        nc, in_maps, core_ids=list(range(NCORES)), trace=trace)
    y = np.asarray(res.results[0]["out"], np.float32).reshape(1)
    return y, res


def kernel(**inputs) -> np.ndarray:
    y, _ = run(**inputs)
    return y



# revision 5
# speedup vs baseline: 1.9688x; 1.3171x over previous
"""3-layer GCN (message passing) + sum-pool + MLP head on 8 Trainium2 cores.

Strategy (all shapes hardcoded; self-contained):
  - Host graph preprocessing: permute nodes into 392 blocks of 128 (49
    blocks/core); nodes split into two gather halves (int16 idx limit) with
    per-block-half edge capacity (1152/1024) balanced by a greedy packer.
  - Layer 1 aggregates x*d_inv directly (linearity: A(xW0) = (Ax)W0), so the
    L1 gather table is a replicated input - no dense pre-pass, no AllGather.
  - Tables are bf16 [*, 128]; one-hot routing matrices are exact 0/1 bf16
    built by DVE from bf16 iota/dstloc (2x DVE rate); all GCN normalization
    is folded into d_inv^2 epilogues on the dense outputs (commutes with
    relu since d_inv > 0). Self-loops are a constant identity-matmul chunk.
    L3 folds d_inv[dst] into its one-hot values and sum-pools via relu
    accum_out.
  - Edge gathers run on 4 SWDGE queues (disjoint Q7 core pairs) so up to 4
    descriptor generations proceed concurrently; 10 small batches with
    triple-buffered gather tiles keep the queues fed.
  - Node-sharded dense outputs are AllGather'd between layers; pooled vector
    is AllReduce'd; the tiny MLP head runs replicated on-device.
"""
import sys

import numpy as np

for _p in ("/opt/trn_rl_repo", "/root/.axon_site/_ro/trn_rl_repo"):
    if _p not in sys.path:
        sys.path.append(_p)

import ml_dtypes

import concourse.bacc as bacc
import concourse.bass as bass
import concourse.mybir as mybir
import concourse.tile as tile
from concourse.bass_utils import run_bass_kernel_spmd

# ---------------------------------------------------------------- constants
N = 50000                 # real nodes
P = 128
NB = 392                  # blocks (of 128 node slots)
NP = NB * P               # padded nodes = 50176
NCORES = 8
BPC = NB // NCORES        # 49 blocks per core
ROWS_PC = BPC * P         # 6272 rows per core shard
NHB = NB // 2             # 196 blocks per half
H = NHB * P               # 25088 = gather-half split (int16 idx limit)
CAPA = 1152               # edge capacity per block, source-half A
CAPB = 1024               # edge capacity per block, source-half B
NCHA = CAPA // P          # 9 chunks from half A
NCHB = CAPB // P          # 8 chunks from half B
CHB = NCHA + NCHB         # 17 edge chunks per block
BATCH_SIZES = [5] * 9 + [4]          # gather batching of the 49 blocks
IDX_COLS = ((CAPA + CAPB) // 16) * BPC  # 6664 idx columns (int16, wrap 16)
FW = 128                  # stored table width (bf16)

_CACHED_NC = None
BF16 = ml_dtypes.bfloat16


# ------------------------------------------------------------- host prepro
def _balance_blocks(a_w, b_w, nblocks, cap_a, cap_b):
    """Greedy-pack nodes (with per-node loads a_w/b_w) into blocks of <=128
    nodes with per-half loads <= cap. Returns block id per node position."""
    order = np.argsort(-(a_w + b_w), kind="stable")
    la = np.zeros(nblocks, np.int64)
    lb = np.zeros(nblocks, np.int64)
    cnt = np.zeros(nblocks, np.int64)
    out = np.empty(len(a_w), np.int64)
    for i in order:
        na = la + a_w[i]
        nb_ = lb + b_w[i]
        score = np.maximum(na / cap_a, nb_ / cap_b)
        score[(cnt >= P) | (na > cap_a) | (nb_ > cap_b)] = np.inf
        j = int(np.argmin(score))
        assert np.isfinite(score[j]), "block packing infeasible; raise CAP"
        out[i] = j
        la[j] = na[j]
        lb[j] = nb_[j]
        cnt[j] += 1
    return out


def _preprocess(x, edge_index):
    src = np.asarray(edge_index[0], np.int64)
    dst = np.asarray(edge_index[1], np.int64)

    deg = np.bincount(dst, minlength=N).astype(np.float64)
    d_inv = 1.0 / np.sqrt(deg + 1.0)

    # ---- split nodes into halves; bias out-edge mass toward half A's
    # larger capacity
    targ_a = CAPA / (CAPA + CAPB)
    out_w = np.bincount(src, minlength=N)
    order = np.argsort(-out_w, kind="stable")
    half = np.zeros(N, np.int8)
    tot = [0.0, 0.0]
    cnti = [0, 0]
    for i in order:
        fa = tot[0] / targ_a
        fb = tot[1] / (1.0 - targ_a)
        h_ = 0 if (fa <= fb and cnti[0] < H) or cnti[1] >= H else 1
        half[i] = h_
        tot[h_] += out_w[i]
        cnti[h_] += 1

    # ---- per-node in-loads split by source half
    sh = half[src]
    a_in = np.bincount(dst[sh == 0], minlength=N)
    b_in = np.bincount(dst[sh == 1], minlength=N)

    # ---- pack each half's nodes into its 196 blocks
    perm_pos = np.empty(N, np.int64)  # node -> permuted id
    for h_ in (0, 1):
        nodes = np.nonzero(half == h_)[0]
        blk = _balance_blocks(a_in[nodes], b_in[nodes], NHB, CAPA, CAPB)
        o2 = np.argsort(blk, kind="stable")
        sb = blk[o2]
        grp_start = np.searchsorted(sb, np.arange(NHB), side="left")
        pos_in_grp = np.arange(len(nodes)) - grp_start[sb]
        perm_pos[nodes[o2]] = (h_ * NHB + sb) * P + pos_in_grp

    # ---- remap edges, group by (dst block, src half)
    psrc = perm_pos[src]
    pdst = perm_pos[dst]
    eb = pdst >> 7              # dst block
    es = pdst & 127             # dst slot
    eh = (psrc >= H).astype(np.int64)
    eidx = psrc - eh * H        # gather idx within half

    key = eb * 2 + eh
    order_e = np.argsort(key, kind="stable")
    key_s = key[order_e]
    cnts = np.bincount(key_s, minlength=NB * 2)
    cap_arr = np.where(np.arange(NB * 2) % 2 == 0, CAPA, CAPB)
    assert (cnts <= cap_arr).all(), "block-half overflow; raise CAP"
    starts = np.concatenate([[0], np.cumsum(cnts)[:-1]])
    pos = np.arange(len(key_s)) - starts[key_s]

    # ---- fill per-core device arrays
    idxs = np.zeros((NCORES, 16, IDX_COLS), np.int16)
    dstloc = np.full((NCORES, P, BPC * CHB), 999.0, BF16)
    w3vals = np.zeros((NCORES, P, BPC * CHB), BF16)

    g_eb = eb[order_e]
    g_eh = eh[order_e]
    g_core = g_eb // BPC
    g_j = g_eb % BPC                      # core-local block
    bs_arr = np.array(BATCH_SIZES)
    blk2batch = np.repeat(np.arange(len(bs_arr)), bs_arr)
    batch_blk0 = np.concatenate([[0], np.cumsum(bs_arr)[:-1]])
    g_batch = blk2batch[g_j]
    g_k = g_j - batch_blk0[g_batch]       # block within batch

    # dstloc / L3 one-hot values: col = j*17 + h*9 + pos//128, row = pos%128
    col_dw = g_j * CHB + g_eh * NCHA + pos // P
    dstloc[g_core, pos % P, col_dw] = es[order_e]
    w3vals[g_core, pos % P, col_dw] = d_inv[dst[order_e]]

    # idx: batch-grouped wrapped layout; call (t, A) then (t, B)
    batch_col0 = np.concatenate(
        [[0], np.cumsum(((CAPA + CAPB) // 16) * bs_arr)[:-1]])
    cap_eh = np.where(g_eh == 0, CAPA, CAPB)
    call_off = batch_col0[g_batch] + g_eh * (CAPA // 16) * bs_arr[g_batch]
    q = g_k * cap_eh + pos
    idxs[g_core, q % 16, call_off + q // 16] = eidx[order_e]
    idxs_full = np.tile(idxs, (1, 8, 1))  # replicate to 128 partitions

    # ---- bf16 L1 gather table: xg[perm(n), 0:14] = x[n] * d_inv[n]
    xg = np.zeros((NP, FW), BF16)
    xg[perm_pos, :14] = (np.asarray(x, np.float64)
                         * d_inv[:, None]).astype(BF16)
    xg_own = xg.reshape(NCORES, ROWS_PC, FW)

    # ---- per-slot d_inv arrays [core][slot, block]
    dinv1 = np.zeros((NCORES, P, BPC), np.float32)
    dinv2 = np.zeros((NCORES, P, BPC), np.float32)
    blk_all = perm_pos >> 7
    slot_all = perm_pos & 127
    dinv1[blk_all // BPC, slot_all, blk_all % BPC] = d_inv
    dinv2[blk_all // BPC, slot_all, blk_all % BPC] = d_inv * d_inv
    return xg, xg_own, idxs_full, dstloc, w3vals, dinv1, dinv2


# ------------------------------------------------------------ device build
def _build_kernel():
    nc = bacc.Bacc("TRN2", target_bir_lowering=False, debug=False,
                   num_swdge_queues=4)
    dt = mybir.dt

    xg = nc.dram_tensor("xg", [NP, FW], dt.bfloat16, kind="ExternalInput")
    xgo = nc.dram_tensor("xgo", [ROWS_PC, FW], dt.bfloat16, kind="ExternalInput")
    w0 = nc.dram_tensor("w0", [14, 128], dt.float32, kind="ExternalInput")
    w1 = nc.dram_tensor("w1", [128, 128], dt.float32, kind="ExternalInput")
    w2p = nc.dram_tensor("w2p", [128, FW], dt.float32, kind="ExternalInput")
    fc11w = nc.dram_tensor("fc11w", [32, 16], dt.float32, kind="ExternalInput")
    fc11b = nc.dram_tensor("fc11b", [16, 1], dt.float32, kind="ExternalInput")
    fc12w = nc.dram_tensor("fc12w", [16, 1], dt.float32, kind="ExternalInput")
    fc12b = nc.dram_tensor("fc12b", [1, 1], dt.float32, kind="ExternalInput")
    iota = nc.dram_tensor("iota", [P, P], dt.bfloat16, kind="ExternalInput")
    iotab = nc.dram_tensor("iotab", [P, CHB * P], dt.bfloat16, kind="ExternalInput")
    ident = nc.dram_tensor("ident", [P, P], dt.bfloat16, kind="ExternalInput")
    prange = nc.dram_tensor("prange", [P, 1], dt.float32, kind="ExternalInput")
    dinv1 = nc.dram_tensor("dinv1", [P, BPC], dt.float32, kind="ExternalInput")
    dinv2 = nc.dram_tensor("dinv2", [P, BPC], dt.float32, kind="ExternalInput")
    idxs = nc.dram_tensor("idxs", [P, IDX_COLS], dt.int16, kind="ExternalInput")
    dstloc = nc.dram_tensor("dstloc", [P, BPC * CHB], dt.bfloat16, kind="ExternalInput")
    w3v = nc.dram_tensor("w3v", [P, BPC * CHB], dt.bfloat16, kind="ExternalInput")
    out = nc.dram_tensor("out", [1, 1], dt.float32, kind="ExternalOutput")

    bs_arr = np.array(BATCH_SIZES)
    batch_col0 = np.concatenate(
        [[0], np.cumsum(((CAPA + CAPB) // 16) * bs_arr)[:-1]])
    batch_blk0 = np.concatenate([[0], np.cumsum(bs_arr)[:-1]])

    with tile.TileContext(nc) as tc:
        with (
            tc.tile_pool(name="const", bufs=1) as cst,
            tc.tile_pool(name="ga", bufs=3) as gap,
            tc.tile_pool(name="gb", bufs=3) as gbp,
            tc.tile_pool(name="oh", bufs=8) as ohp,
            tc.tile_pool(name="eq", bufs=2) as eqp,
            tc.tile_pool(name="rl", bufs=3) as rlp,
            tc.tile_pool(name="st", bufs=2) as stp,
            tc.tile_pool(name="misc", bufs=1) as msc,
            tc.tile_pool(name="psA", bufs=2, space="PSUM") as psa,
            tc.tile_pool(name="psX", bufs=2, space="PSUM") as psx,
            tc.tile_pool(name="psD", bufs=2, space="PSUM") as psd,
            tc.tile_pool(name="psP", bufs=1, space="PSUM") as psp,
            tc.tile_pool(name="dram", bufs=1, space="DRAM") as drm,
        ):
            # resident constants
            idxs_t = cst.tile([P, IDX_COLS], dt.int16)
            dstloc_t = cst.tile([P, BPC * CHB], dt.bfloat16)
            w3v_t = cst.tile([P, BPC * CHB], dt.bfloat16)
            iota_t = cst.tile([P, P], dt.bfloat16)
            iotab_t = cst.tile([P, CHB * P], dt.bfloat16)
            ident_t = cst.tile([P, P], dt.bfloat16)
            prange_t = cst.tile([P, 1], dt.float32)
            dinv1_t = cst.tile([P, BPC], dt.float32)
            dinv2_t = cst.tile([P, BPC], dt.float32)
            w0_t = cst.tile([14, 128], dt.float32)
            w1_t = cst.tile([128, 128], dt.float32)
            w2p_t = cst.tile([128, FW], dt.float32)
            fc11w_t = cst.tile([32, 16], dt.float32)
            fc11b_t = cst.tile([16, 1], dt.float32)
            fc12w_t = cst.tile([16, 1], dt.float32)
            fc12b_t = cst.tile([1, 1], dt.float32)
            for t_, d_ in (
                (idxs_t, idxs), (dstloc_t, dstloc), (w3v_t, w3v),
                (iota_t, iota), (iotab_t, iotab),
                (ident_t, ident), (prange_t, prange),
                (dinv1_t, dinv1), (dinv2_t, dinv2),
                (w0_t, w0), (w1_t, w1), (w2p_t, w2p),
                (fc11w_t, fc11w), (fc11b_t, fc11b), (fc12w_t, fc12w),
                (fc12b_t, fc12b),
            ):
                nc.sync.dma_start(t_[:], d_[:])

            # internal DRAM (bf16 tables)
            g2s_t = drm.tile([ROWS_PC, FW], dt.bfloat16)
            g2_t = drm.tile([NP, FW], dt.bfloat16)
            g3s_t = drm.tile([ROWS_PC, FW], dt.bfloat16)
            g3_t = drm.tile([NP, FW], dt.bfloat16)
            pool_in = drm.tile([32, 1], dt.float32)
            pool_out = drm.tile([32, 1], dt.float32)

            pooled_cols = msc.tile([32, BPC], dt.float32)

            def layer(lnum, h_src, h_self, h_shard):
                for t, bs in enumerate(BATCH_SIZES):
                    ic0 = int(batch_col0[t])
                    icb0 = ic0 + (CAPA // 16) * bs
                    b0 = int(batch_blk0[t])
                    ga = gap.tile([P, NCHA * 5, FW], dt.bfloat16, tag="ga")
                    gb = gbp.tile([P, NCHB * 5, FW], dt.bfloat16, tag="gb")
                    nc.gpsimd.dma_gather(
                        ga[:, : NCHA * bs, :], h_src[0:H, :],
                        idxs_t[:, ic0:icb0],
                        CAPA * bs, CAPA * bs, FW, single_packet=False,
                        queue_num=(2 * t) % 4)
                    nc.gpsimd.dma_gather(
                        gb[:, : NCHB * bs, :], h_src[H:NP, :],
                        idxs_t[:, icb0 : icb0 + (CAPB // 16) * bs],
                        CAPB * bs, CAPB * bs, FW, single_packet=False,
                        queue_num=(2 * t + 1) % 4)
                    gs = gap.tile([P, 5, FW], dt.bfloat16, tag="gs")
                    nc.sync.dma_start(
                        gs[:, :bs, :],
                        h_self[b0 * P : (b0 + bs) * P, :].rearrange(
                            "(g p) f -> p g f", p=P))
                    if lnum != 3:
                        dstage = stp.tile([P, 5, FW], dt.bfloat16, tag="dnst")
                    for k in range(bs):
                        j = b0 + k
                        agg = psa.tile([P, P], dt.float32, tag="agg")
                        # build all 17 one-hot chunks of this block at once
                        iview = iotab_t[:].rearrange("p (c j) -> p c j", j=P)
                        dview = dstloc_t[:, j * CHB : (j + 1) * CHB].rearrange(
                            "p (c o) -> p c o", o=1).to_broadcast([P, CHB, P])
                        ohb = ohp.tile([P, CHB, P], dt.bfloat16, tag="ohb")
                        if lnum == 3:
                            eqt = eqp.tile([P, CHB, P], dt.bfloat16, tag="eqt")
                            nc.vector.tensor_tensor(
                                eqt[:], iview, dview, mybir.AluOpType.is_equal)
                            wview = w3v_t[:, j * CHB : (j + 1) * CHB].rearrange(
                                "p (c o) -> p c o", o=1).to_broadcast(
                                    [P, CHB, P])
                            nc.vector.tensor_tensor(
                                ohb[:], eqt[:], wview, mybir.AluOpType.mult)
                        else:
                            nc.vector.tensor_tensor(
                                ohb[:], iview, dview, mybir.AluOpType.is_equal)
                        for c in range(CHB):
                            if c < NCHA:
                                g = ga
                                cc = NCHA * k + c
                            else:
                                g = gb
                                cc = NCHB * k + (c - NCHA)
                            nc.tensor.matmul(
                                agg[:], g[:, cc, :], ohb[:, c, :],
                                start=(c == 0), stop=False)
                        # self-loop chunk: identity (L1/L2) or d_inv diagonal
                        if lnum == 3:
                            ohs = ohp.tile([P, P], dt.bfloat16, tag="ohs")
                            nc.vector.tensor_scalar(
                                ohs[:], iota_t[:],
                                prange_t[:, 0:1], dinv1_t[:, j : j + 1],
                                mybir.AluOpType.is_equal, mybir.AluOpType.mult)
                            nc.tensor.matmul(
                                agg[:], gs[:, k, :], ohs[:],
                                start=False, stop=True)
                        else:
                            nc.tensor.matmul(
                                agg[:], gs[:, k, :], ident_t[:],
                                start=False, stop=True)

                        if lnum == 1:
                            # ZT = W0^T @ agg_x[0:14]; relu; dense W1; epilogue
                            axs = rlp.tile([14, P], dt.float32, tag="axs")
                            nc.vector.tensor_copy(axs[:], agg[0:14, :])
                            zt = psx.tile([P, P], dt.float32, tag="zt")
                            nc.tensor.matmul(
                                zt[:], w0_t[:], axs[:], start=True, stop=True)
                            rT = rlp.tile([P, P], dt.float32, tag="rT")
                            nc.scalar.activation(
                                rT[:], zt[:],
                                mybir.ActivationFunctionType.Relu)
                            h_ps = psd.tile([P, FW], dt.float32, tag="dnps")
                            nc.tensor.matmul(
                                h_ps[:], rT[:], w1_t[:], start=True, stop=True)
                            nc.vector.tensor_scalar(
                                dstage[:, k, :], h_ps[:],
                                dinv2_t[:, j : j + 1], None,
                                mybir.AluOpType.mult)
                        elif lnum == 2:
                            rT = rlp.tile([P, P], dt.float32, tag="rT")
                            nc.scalar.activation(
                                rT[:], agg[:],
                                mybir.ActivationFunctionType.Relu)
                            h_ps = psd.tile([P, FW], dt.float32, tag="dnps")
                            nc.tensor.matmul(
                                h_ps[:], rT[:], w2p_t[:], start=True, stop=True)
                            nc.vector.tensor_scalar(
                                dstage[:, k, :], h_ps[:],
                                dinv2_t[:, j : j + 1], None,
                                mybir.AluOpType.mult)
                        else:
                            # L3: out3T = relu(agg[0:32]); pooled col = row sum
                            r3 = rlp.tile([32, P], dt.float32, tag="r3")
                            nc.scalar.activation(
                                r3[:], agg[0:32, :],
                                mybir.ActivationFunctionType.Relu,
                                accum_out=pooled_cols[:, j : j + 1])
                    if lnum != 3:
                        r0 = b0 * P
                        nc.sync.dma_start(
                            h_shard[r0 : r0 + bs * P, :].rearrange(
                                "(g p) f -> p g f", p=P),
                            dstage[:, :bs, :])

            # L1 (x-aggregation)
            layer(1, xg, xgo, g2s_t)
            nc.gpsimd.collective_compute(
                "AllGather", mybir.AluOpType.bypass,
                replica_groups=[list(range(NCORES))],
                ins=[g2s_t.opt()], outs=[g2_t.opt()])
            # L2
            layer(2, g2_t, g2s_t, g3s_t)
            nc.gpsimd.collective_compute(
                "AllGather", mybir.AluOpType.bypass,
                replica_groups=[list(range(NCORES))],
                ins=[g3s_t.opt()], outs=[g3_t.opt()])
            # L3 + pooling
            layer(3, g3_t, g3s_t, None)
            pooled = msc.tile([32, 1], dt.float32)
            nc.vector.tensor_reduce(
                pooled[:], pooled_cols[:],
                axis=mybir.AxisListType.X, op=mybir.AluOpType.add)

            # global pool AllReduce + MLP head (replicated)
            nc.sync.dma_start(pool_in[:], pooled[:])
            nc.gpsimd.collective_compute(
                "AllReduce", mybir.AluOpType.add,
                replica_groups=[list(range(NCORES))],
                ins=[pool_in.opt()], outs=[pool_out.opt()])
            pooled_g = msc.tile([32, 1], dt.float32)
            nc.sync.dma_start(pooled_g[:], pool_out[:])
            ps16 = psp.tile([16, 1], dt.float32, tag="mlp")
            nc.tensor.matmul(ps16[:], fc11w_t[:], pooled_g[:], start=True, stop=True)
            a16 = msc.tile([16, 1], dt.float32)
            nc.scalar.activation(
                a16[:], ps16[:], mybir.ActivationFunctionType.Relu,
                bias=fc11b_t[:])
            ps1 = psp.tile([1, 1], dt.float32, tag="mlp")
            nc.tensor.matmul(ps1[:], fc12w_t[:], a16[:], start=True, stop=True)
            o1 = msc.tile([1, 1], dt.float32)
            nc.scalar.activation(
                o1[:], ps1[:], mybir.ActivationFunctionType.Identity,
                bias=fc12b_t[:])
            nc.sync.dma_start(out[:], o1[:])

    nc.compile()
    return nc


def _get_nc():
    global _CACHED_NC
    if _CACHED_NC is None:
        _CACHED_NC = _build_kernel()
    return _CACHED_NC


def _make_in_maps(inputs):
    x = np.asarray(inputs["x"], np.float32)
    edge_index = np.asarray(inputs["edge_index"])
    xg, xg_own, idxs, dstloc, w3vals, dinv1, dinv2 = _preprocess(x, edge_index)

    w2p = np.zeros((128, FW), np.float32)
    w2p[:, :32] = np.asarray(inputs["W2"], np.float32)
    common = {
        "xg": xg,
        "w0": np.asarray(inputs["W0"], np.float32),
        "w1": np.asarray(inputs["W1"], np.float32),
        "w2p": w2p,
        "fc11w": np.asarray(inputs["fc11_w"], np.float32),
        "fc11b": np.asarray(inputs["fc11_b"], np.float32).reshape(16, 1),
        "fc12w": np.asarray(inputs["fc12_w"], np.float32),
        "fc12b": np.asarray(inputs["fc12_b"], np.float32).reshape(1, 1),
        "iota": np.tile(np.arange(P, dtype=np.float32),
                        (P, 1)).astype(BF16),
        "iotab": np.tile(np.arange(P, dtype=np.float32),
                         (P, CHB)).astype(BF16),
        "ident": np.eye(P, dtype=BF16),
        "prange": np.arange(P, dtype=np.float32).reshape(P, 1),
    }
    return [
        {**common, "xgo": np.ascontiguousarray(xg_own[c]), "idxs": idxs[c],
         "dstloc": dstloc[c], "w3v": w3vals[c],
         "dinv1": dinv1[c], "dinv2": dinv2[c]}
        for c in range(NCORES)
    ]


def run(trace=False, _inputs=None, **inputs):
    if _inputs is not None:
        inputs = _inputs
    in_maps = _make_in_maps(inputs)
    nc = _get_nc()
    res = run_bass_kernel_spmd(
        nc, in_maps, core_ids=list(range(NCORES)), trace=trace)
    y = np.asarray(res.results[0]["out"], np.float32).reshape(1)
    return y, res


def kernel(**inputs) -> np.ndarray:
    y, _ = run(**inputs)
    return y


# revision 7
# speedup vs baseline: 2.1746x; 1.1045x over previous
"""3-layer GCN (message passing) + sum-pool + MLP head on 8 Trainium2 cores.

Strategy (all shapes hardcoded; self-contained):
  - Host graph preprocessing: permute nodes into 392 blocks of 128; each core
    owns 25 "half A" blocks + 24 "half B" blocks (the halves are the two
    gather tables, int16 idx limit). Per-block edge capacity 1152 (A-sourced)
    + 1024 (B-sourced), balanced by a greedy packer.
  - Layer 1 aggregates x*d_inv directly (linearity: A(xW0) = (Ax)W0), so the
    L1 gather table is a replicated input - no dense pre-pass, no AllGather.
  - Tables are bf16 [*, 128]. One-hot routing matrices (plain 0/1 for L1/L2,
    d_inv[dst]-weighted for L3) are HOST-precomputed and streamed in by DMA,
    so no engine builds them. GCN normalization folds into d_inv^2 epilogues
    (commutes with relu since d_inv > 0). Self-loops are a constant
    identity-matmul chunk; L3's self-loop uses a d_inv diagonal built by DVE.
  - Edge gathers run on 4 SWDGE queues (disjoint GpSimd Q7 core pairs) so up
    to 4 descriptor generations proceed concurrently; 10 small batches with
    4-deep gather tile pools keep the queues fed.
  - The inter-layer AllGather is split in two chunks (half A = local blocks
    0-24, half B = 25-48) so the A chunk overlaps the tail of the producing
    layer; pooled vector is AllReduce'd; tiny MLP head runs replicated.
"""
import sys

import numpy as np

for _p in ("/opt/trn_rl_repo", "/root/.axon_site/_ro/trn_rl_repo"):
    if _p not in sys.path:
        sys.path.append(_p)

import ml_dtypes

import concourse.bacc as bacc
import concourse.bass as bass
import concourse.mybir as mybir
import concourse.tile as tile
from concourse.bass_utils import run_bass_kernel_spmd

# ---------------------------------------------------------------- constants
N = 50000                 # real nodes
P = 128
NCORES = 8
ABPC = 25                 # half-A blocks per core
BBPC = 24                 # half-B blocks per core
BPC = ABPC + BBPC         # 49 blocks per core
NB = BPC * NCORES         # 392 blocks
NP = NB * P               # padded nodes = 50176
ROWS_PC = BPC * P         # 6272 rows per core shard
HA = ABPC * NCORES * P    # 25600 rows in gather-half A
HB = NP - HA              # 24576 rows in gather-half B
CAPA = 1152               # edge capacity per block, source-half A
CAPB = 1024               # edge capacity per block, source-half B
NCHA = CAPA // P          # 9 chunks from half A
NCHB = CAPB // P          # 8 chunks from half B
CHB = NCHA + NCHB         # 17 edge chunks per block
BATCH_SIZES = [5] * 9 + [4]          # gather batching of the 49 blocks
IDX_COLS = ((CAPA + CAPB) // 16) * BPC  # 6664 idx columns (int16, wrap 16)
FW = 128                  # stored table width (bf16)

_CACHED_NC = None
BF16 = ml_dtypes.bfloat16


# ------------------------------------------------------------- host prepro
def _balance_blocks(a_w, b_w, nblocks, cap_a, cap_b):
    """Greedy-pack nodes (with per-node loads a_w/b_w) into blocks of <=128
    nodes with per-half loads <= cap. Returns block id per node position."""
    order = np.argsort(-(a_w + b_w), kind="stable")
    la = np.zeros(nblocks, np.int64)
    lb = np.zeros(nblocks, np.int64)
    cnt = np.zeros(nblocks, np.int64)
    out = np.empty(len(a_w), np.int64)
    for i in order:
        na = la + a_w[i]
        nb_ = lb + b_w[i]
        score = np.maximum(na / cap_a, nb_ / cap_b)
        score[(cnt >= P) | (na > cap_a) | (nb_ > cap_b)] = np.inf
        j = int(np.argmin(score))
        assert np.isfinite(score[j]), "block packing infeasible; raise CAP"
        out[i] = j
        la[j] = na[j]
        lb[j] = nb_[j]
        cnt[j] += 1
    return out


def _preprocess(x, edge_index):
    src = np.asarray(edge_index[0], np.int64)
    dst = np.asarray(edge_index[1], np.int64)

    deg = np.bincount(dst, minlength=N).astype(np.float64)
    d_inv = 1.0 / np.sqrt(deg + 1.0)

    # ---- split nodes into halves; bias out-edge mass toward half A's
    # larger capacity
    targ_a = CAPA / (CAPA + CAPB)
    out_w = np.bincount(src, minlength=N)
    order = np.argsort(-out_w, kind="stable")
    half = np.zeros(N, np.int8)
    tot = [0.0, 0.0]
    cnti = [0, 0]
    for i in order:
        fa = tot[0] / targ_a
        fb = tot[1] / (1.0 - targ_a)
        h_ = 0 if (fa <= fb and cnti[0] < HA) or cnti[1] >= HB else 1
        half[i] = h_
        tot[h_] += out_w[i]
        cnti[h_] += 1

    # ---- per-node in-loads split by source half
    sh = half[src]
    a_in = np.bincount(dst[sh == 0], minlength=N)
    b_in = np.bincount(dst[sh == 1], minlength=N)

    # ---- pack each half's nodes into its blocks; round-robin blocks over
    # cores. Global table row layout: half A rows [0, HA) are core-major
    # (core c -> rows [c*ABPC*P, (c+1)*ABPC*P)); half B rows [HA, NP)
    # likewise with BBPC. A core's shard is its A rows ++ its B rows.
    perm_pos = np.empty(N, np.int64)  # node -> global table row
    for h_, nhb, bpc_h, base in (
        (0, ABPC * NCORES, ABPC, 0),
        (1, BBPC * NCORES, BBPC, HA),
    ):
        nodes = np.nonzero(half == h_)[0]
        blk = _balance_blocks(a_in[nodes], b_in[nodes], nhb, CAPA, CAPB)
        o2 = np.argsort(blk, kind="stable")
        sb = blk[o2]
        grp_start = np.searchsorted(sb, np.arange(nhb), side="left")
        pos_in_grp = np.arange(len(nodes)) - grp_start[sb]
        core = sb % NCORES
        jloc = sb // NCORES       # block index within the core's half part
        perm_pos[nodes[o2]] = (
            base + (core * bpc_h + jloc) * P + pos_in_grp)

    # ---- remap edges; (core, local block) of each dst row
    psrc = perm_pos[src]
    pdst = perm_pos[dst]

    def row_to_cj(rows):
        in_a = rows < HA
        ra = rows
        rb = rows - HA
        c = np.where(in_a, ra // (ABPC * P), rb // (BBPC * P))
        j = np.where(in_a, (ra % (ABPC * P)) // P,
                     ABPC + (rb % (BBPC * P)) // P)
        return c, j

    dc, dj = row_to_cj(pdst)
    es = pdst % P               # dst slot
    eh = (psrc >= HA).astype(np.int64)
    eidx = psrc - eh * HA       # gather idx within half

    key = (dc * BPC + dj) * 2 + eh
    order_e = np.argsort(key, kind="stable")
    key_s = key[order_e]
    cnts = np.bincount(key_s, minlength=NB * 2)
    cap_arr = np.where(np.arange(NB * 2) % 2 == 0, CAPA, CAPB)
    assert (cnts <= cap_arr).all(), "block-half overflow; raise CAP"
    starts = np.concatenate([[0], np.cumsum(cnts)[:-1]])
    pos = np.arange(len(key_s)) - starts[key_s]

    # ---- fill per-core device arrays
    idxs = np.zeros((NCORES, 16, IDX_COLS), np.int16)
    oh1 = np.zeros((NCORES, P, BPC * CHB, P), BF16)
    oh3 = np.zeros((NCORES, P, BPC * CHB, P), BF16)

    g_core = dc[order_e]
    g_j = dj[order_e]
    g_eh = eh[order_e]
    bs_arr = np.array(BATCH_SIZES)
    blk2batch = np.repeat(np.arange(len(bs_arr)), bs_arr)
    batch_blk0 = np.concatenate([[0], np.cumsum(bs_arr)[:-1]])
    g_batch = blk2batch[g_j]
    g_k = g_j - batch_blk0[g_batch]       # block within batch

    # one-hot tables: col = j*17 + h*9 + pos//128, row = pos%128, val col es
    col_dw = g_j * CHB + g_eh * NCHA + pos // P
    oh1[g_core, pos % P, col_dw, es[order_e]] = 1
    oh3[g_core, pos % P, col_dw, es[order_e]] = d_inv[dst[order_e]]

    # idx: batch-grouped wrapped layout; call (t, A) then (t, B)
    batch_col0 = np.concatenate(
        [[0], np.cumsum(((CAPA + CAPB) // 16) * bs_arr)[:-1]])
    cap_eh = np.where(g_eh == 0, CAPA, CAPB)
    call_off = batch_col0[g_batch] + g_eh * (CAPA // 16) * bs_arr[g_batch]
    q = g_k * cap_eh + pos
    idxs[g_core, q % 16, call_off + q // 16] = eidx[order_e]
    idxs_full = np.tile(idxs, (1, 8, 1))  # replicate to 128 partitions

    # ---- bf16 L1 gather table: xg[perm(n), 0:14] = x[n] * d_inv[n]
    xg = np.zeros((NP, FW), BF16)
    xg[perm_pos, :14] = (np.asarray(x, np.float64)
                         * d_inv[:, None]).astype(BF16)
    xg_own = np.concatenate([
        xg[: HA].reshape(NCORES, ABPC * P, FW),
        xg[HA:].reshape(NCORES, BBPC * P, FW)], axis=1)

    # ---- per-slot d_inv arrays [core][slot, block]
    dinv1 = np.zeros((NCORES, P, BPC), np.float32)
    dinv2 = np.zeros((NCORES, P, BPC), np.float32)
    ac, aj = row_to_cj(perm_pos)
    dinv1[ac, perm_pos % P, aj] = d_inv
    dinv2[ac, perm_pos % P, aj] = d_inv * d_inv
    return xg, xg_own, idxs_full, oh1, oh3, dinv1, dinv2


# ------------------------------------------------------------ device build
def _build_kernel():
    nc = bacc.Bacc("TRN2", target_bir_lowering=False, debug=False,
                   num_swdge_queues=4)
    dt = mybir.dt

    xg = nc.dram_tensor("xg", [NP, FW], dt.bfloat16, kind="ExternalInput")
    xgo = nc.dram_tensor("xgo", [ROWS_PC, FW], dt.bfloat16, kind="ExternalInput")
    w0 = nc.dram_tensor("w0", [14, 128], dt.float32, kind="ExternalInput")
    w1 = nc.dram_tensor("w1", [128, 128], dt.float32, kind="ExternalInput")
    w2p = nc.dram_tensor("w2p", [128, FW], dt.float32, kind="ExternalInput")
    fc11w = nc.dram_tensor("fc11w", [32, 16], dt.float32, kind="ExternalInput")
    fc11b = nc.dram_tensor("fc11b", [16, 1], dt.float32, kind="ExternalInput")
    fc12w = nc.dram_tensor("fc12w", [16, 1], dt.float32, kind="ExternalInput")
    fc12b = nc.dram_tensor("fc12b", [1, 1], dt.float32, kind="ExternalInput")
    iota = nc.dram_tensor("iota", [P, P], dt.float32, kind="ExternalInput")
    ident = nc.dram_tensor("ident", [P, P], dt.bfloat16, kind="ExternalInput")
    prange = nc.dram_tensor("prange", [P, 1], dt.float32, kind="ExternalInput")
    dinv1 = nc.dram_tensor("dinv1", [P, BPC], dt.float32, kind="ExternalInput")
    dinv2 = nc.dram_tensor("dinv2", [P, BPC], dt.float32, kind="ExternalInput")
    idxs = nc.dram_tensor("idxs", [P, IDX_COLS], dt.int16, kind="ExternalInput")
    oh1 = nc.dram_tensor("oh1", [P, BPC * CHB * P], dt.bfloat16,
                         kind="ExternalInput")
    oh3 = nc.dram_tensor("oh3", [P, BPC * CHB * P], dt.bfloat16,
                         kind="ExternalInput")
    out = nc.dram_tensor("out", [1, 1], dt.float32, kind="ExternalOutput")

    bs_arr = np.array(BATCH_SIZES)
    batch_col0 = np.concatenate(
        [[0], np.cumsum(((CAPA + CAPB) // 16) * bs_arr)[:-1]])
    batch_blk0 = np.concatenate([[0], np.cumsum(bs_arr)[:-1]])

    with tile.TileContext(nc) as tc:
        with (
            tc.tile_pool(name="const", bufs=1) as cst,
            tc.tile_pool(name="ga", bufs=4) as gap,
            tc.tile_pool(name="gb", bufs=4) as gbp,
            tc.tile_pool(name="oh", bufs=3) as ohp,
            tc.tile_pool(name="ohs", bufs=2) as ohsp,
            tc.tile_pool(name="rl", bufs=3) as rlp,
            tc.tile_pool(name="st", bufs=2) as stp,
            tc.tile_pool(name="misc", bufs=1) as msc,
            tc.tile_pool(name="psA", bufs=2, space="PSUM") as psa,
            tc.tile_pool(name="psX", bufs=2, space="PSUM") as psx,
            tc.tile_pool(name="psD", bufs=2, space="PSUM") as psd,
            tc.tile_pool(name="psP", bufs=1, space="PSUM") as psp,
            tc.tile_pool(name="dram", bufs=1, space="DRAM") as drm,
        ):
            # resident constants
            idxs_t = cst.tile([P, IDX_COLS], dt.int16)
            iota_t = cst.tile([P, P], dt.float32)
            ident_t = cst.tile([P, P], dt.bfloat16)
            prange_t = cst.tile([P, 1], dt.float32)
            dinv1_t = cst.tile([P, BPC], dt.float32)
            dinv2_t = cst.tile([P, BPC], dt.float32)
            w0_t = cst.tile([14, 128], dt.float32)
            w1_t = cst.tile([128, 128], dt.float32)
            w2p_t = cst.tile([128, FW], dt.float32)
            fc11w_t = cst.tile([32, 16], dt.float32)
            fc11b_t = cst.tile([16, 1], dt.float32)
            fc12w_t = cst.tile([16, 1], dt.float32)
            fc12b_t = cst.tile([1, 1], dt.float32)
            for t_, d_ in (
                (idxs_t, idxs), (iota_t, iota),
                (ident_t, ident), (prange_t, prange),
                (dinv1_t, dinv1), (dinv2_t, dinv2),
                (w0_t, w0), (w1_t, w1), (w2p_t, w2p),
                (fc11w_t, fc11w), (fc11b_t, fc11b), (fc12w_t, fc12w),
                (fc12b_t, fc12b),
            ):
                nc.sync.dma_start(t_[:], d_[:])

            # internal DRAM (bf16 tables)
            g2s_t = drm.tile([ROWS_PC, FW], dt.bfloat16)
            g2_t = drm.tile([NP, FW], dt.bfloat16)
            g3s_t = drm.tile([ROWS_PC, FW], dt.bfloat16)
            g3_t = drm.tile([NP, FW], dt.bfloat16)
            pool_in = drm.tile([32, 1], dt.float32)
            pool_out = drm.tile([32, 1], dt.float32, addr_space="Shared")

            pooled_cols = msc.tile([32, BPC], dt.float32)

            def ag_half(h_shard, h_full, half):
                if half == 0:
                    i0, i1, o0, o1 = 0, ABPC * P, 0, HA
                else:
                    i0, i1, o0, o1 = ABPC * P, BPC * P, HA, NP
                nc.gpsimd.collective_compute(
                    "AllGather", mybir.AluOpType.bypass,
                    replica_groups=[list(range(NCORES))],
                    ins=[h_shard[i0:i1, :].opt()],
                    outs=[h_full[o0:o1, :].opt()])

            def layer(lnum, h_src, h_self, h_shard, h_full):
                ohsrc = oh3 if lnum == 3 else oh1
                for t, bs in enumerate(BATCH_SIZES):
                    if t == 7 and h_shard is not None:
                        # half-A rows (blocks 0-24, batches 0-4) are staged;
                        # start their AllGather under the remaining batches
                        ag_half(h_shard, h_full, 0)
                    ic0 = int(batch_col0[t])
                    icb0 = ic0 + (CAPA // 16) * bs
                    b0 = int(batch_blk0[t])
                    ga = gap.tile([P, NCHA * 5, FW], dt.bfloat16, tag="ga")
                    gb = gbp.tile([P, NCHB * 5, FW], dt.bfloat16, tag="gb")
                    nc.gpsimd.dma_gather(
                        ga[:, : NCHA * bs, :], h_src[0:HA, :],
                        idxs_t[:, ic0:icb0],
                        CAPA * bs, CAPA * bs, FW, single_packet=False,
                        queue_num=(2 * t) % 4)
                    nc.gpsimd.dma_gather(
                        gb[:, : NCHB * bs, :], h_src[HA:NP, :],
                        idxs_t[:, icb0 : icb0 + (CAPB // 16) * bs],
                        CAPB * bs, CAPB * bs, FW, single_packet=False,
                        queue_num=(2 * t + 1) % 4)
                    gs = gap.tile([P, 5, FW], dt.bfloat16, tag="gs")
                    nc.sync.dma_start(
                        gs[:, :bs, :],
                        h_self[b0 * P : (b0 + bs) * P, :].rearrange(
                            "(g p) f -> p g f", p=P))
                    ohv = ohp.tile([P, 5 * CHB, P], dt.bfloat16, tag="ohv")
                    nc.scalar.dma_start(
                        ohv[:, : bs * CHB, :],
                        ohsrc[:, b0 * CHB * P : (b0 + bs) * CHB * P].rearrange(
                            "p (c j) -> p c j", j=P))
                    if lnum != 3:
                        dstage = stp.tile([P, 5, FW], dt.bfloat16, tag="dnst")
                    for k in range(bs):
                        j = b0 + k
                        agg = psa.tile([P, P], dt.float32, tag="agg")
                        for c in range(CHB):
                            if c < NCHA:
                                g = ga
                                cc = NCHA * k + c
                            else:
                                g = gb
                                cc = NCHB * k + (c - NCHA)
                            nc.tensor.matmul(
                                agg[:], g[:, cc, :], ohv[:, k * CHB + c, :],
                                start=(c == 0), stop=False)
                        # self-loop chunk: identity (L1/L2) or d_inv diagonal
                        if lnum == 3:
                            ohs = ohsp.tile([P, P], dt.bfloat16, tag="ohs")
                            nc.vector.tensor_scalar(
                                ohs[:], iota_t[:],
                                prange_t[:, 0:1], dinv1_t[:, j : j + 1],
                                mybir.AluOpType.is_equal, mybir.AluOpType.mult)
                            nc.tensor.matmul(
                                agg[:], gs[:, k, :], ohs[:],
                                start=False, stop=True)
                        else:
                            nc.tensor.matmul(
                                agg[:], gs[:, k, :], ident_t[:],
                                start=False, stop=True)

                        if lnum == 1:
                            # ZT = W0^T @ agg_x[0:14]; relu; dense W1; epilogue
                            axs = rlp.tile([14, P], dt.float32, tag="axs")
                            nc.vector.tensor_copy(axs[:], agg[0:14, :])
                            zt = psx.tile([P, P], dt.float32, tag="zt")
                            nc.tensor.matmul(
                                zt[:], w0_t[:], axs[:], start=True, stop=True)
                            rT = rlp.tile([P, P], dt.float32, tag="rT")
                            nc.scalar.activation(
                                rT[:], zt[:],
                                mybir.ActivationFunctionType.Relu)
                            h_ps = psd.tile([P, FW], dt.float32, tag="dnps")
                            nc.tensor.matmul(
                                h_ps[:], rT[:], w1_t[:], start=True, stop=True)
                            nc.vector.tensor_scalar(
                                dstage[:, k, :], h_ps[:],
                                dinv2_t[:, j : j + 1], None,
                                mybir.AluOpType.mult)
                        elif lnum == 2:
                            rT = rlp.tile([P, P], dt.float32, tag="rT")
                            nc.scalar.activation(
                                rT[:], agg[:],
                                mybir.ActivationFunctionType.Relu)
                            h_ps = psd.tile([P, FW], dt.float32, tag="dnps")
                            nc.tensor.matmul(
                                h_ps[:], rT[:], w2p_t[:], start=True, stop=True)
                            nc.vector.tensor_scalar(
                                dstage[:, k, :], h_ps[:],
                                dinv2_t[:, j : j + 1], None,
                                mybir.AluOpType.mult)
                        else:
                            # L3: out3T = relu(agg[0:32]); pooled col = row sum
                            r3 = rlp.tile([32, P], dt.float32, tag="r3")
                            nc.scalar.activation(
                                r3[:], agg[0:32, :],
                                mybir.ActivationFunctionType.Relu,
                                accum_out=pooled_cols[:, j : j + 1])
                    if lnum != 3:
                        r0 = b0 * P
                        nc.sync.dma_start(
                            h_shard[r0 : r0 + bs * P, :].rearrange(
                                "(g p) f -> p g f", p=P),
                            dstage[:, :bs, :])
                if h_shard is not None:
                    ag_half(h_shard, h_full, 1)

            # L1 (x-aggregation)
            layer(1, xg, xgo, g2s_t, g2_t)
            # L2
            layer(2, g2_t, g2s_t, g3s_t, g3_t)
            # L3 + pooling
            layer(3, g3_t, g3s_t, None, None)
            pooled = msc.tile([32, 1], dt.float32)
            nc.vector.tensor_reduce(
                pooled[:], pooled_cols[:],
                axis=mybir.AxisListType.X, op=mybir.AluOpType.add)

            # global pool AllReduce + MLP head (replicated)
            nc.sync.dma_start(pool_in[:], pooled[:])
            nc.gpsimd.collective_compute(
                "AllReduce", mybir.AluOpType.add,
                replica_groups=[list(range(NCORES))],
                ins=[pool_in.opt()], outs=[pool_out.opt()])
            pooled_g = msc.tile([32, 1], dt.float32)
            nc.sync.dma_start(pooled_g[:], pool_out[:])
            ps16 = psp.tile([16, 1], dt.float32, tag="mlp")
            nc.tensor.matmul(ps16[:], fc11w_t[:], pooled_g[:], start=True, stop=True)
            a16 = msc.tile([16, 1], dt.float32)
            nc.scalar.activation(
                a16[:], ps16[:], mybir.ActivationFunctionType.Relu,
                bias=fc11b_t[:])
            ps1 = psp.tile([1, 1], dt.float32, tag="mlp")
            nc.tensor.matmul(ps1[:], fc12w_t[:], a16[:], start=True, stop=True)
            o1 = msc.tile([1, 1], dt.float32)
            nc.scalar.activation(
                o1[:], ps1[:], mybir.ActivationFunctionType.Identity,
                bias=fc12b_t[:])
            nc.sync.dma_start(out[:], o1[:])

    nc.compile()
    return nc


def _get_nc():
    global _CACHED_NC
    if _CACHED_NC is None:
        _CACHED_NC = _build_kernel()
    return _CACHED_NC


def _make_in_maps(inputs):
    x = np.asarray(inputs["x"], np.float32)
    edge_index = np.asarray(inputs["edge_index"])
    xg, xg_own, idxs, oh1, oh3, dinv1, dinv2 = _preprocess(x, edge_index)

    w2p = np.zeros((128, FW), np.float32)
    w2p[:, :32] = np.asarray(inputs["W2"], np.float32)
    common = {
        "xg": xg,
        "w0": np.asarray(inputs["W0"], np.float32),
        "w1": np.asarray(inputs["W1"], np.float32),
        "w2p": w2p,
        "fc11w": np.asarray(inputs["fc11_w"], np.float32),
        "fc11b": np.asarray(inputs["fc11_b"], np.float32).reshape(16, 1),
        "fc12w": np.asarray(inputs["fc12_w"], np.float32),
        "fc12b": np.asarray(inputs["fc12_b"], np.float32).reshape(1, 1),
        "iota": np.tile(np.arange(P, dtype=np.float32), (P, 1)),
        "ident": np.eye(P, dtype=BF16),
        "prange": np.arange(P, dtype=np.float32).reshape(P, 1),
    }
    return [
        {**common, "xgo": np.ascontiguousarray(xg_own[c]), "idxs": idxs[c],
         "oh1": oh1[c].reshape(P, BPC * CHB * P),
         "oh3": oh3[c].reshape(P, BPC * CHB * P),
         "dinv1": dinv1[c], "dinv2": dinv2[c]}
        for c in range(NCORES)
    ]


def run(trace=False, _inputs=None, **inputs):
    if _inputs is not None:
        inputs = _inputs
    in_maps = _make_in_maps(inputs)
    nc = _get_nc()
    res = run_bass_kernel_spmd(
        nc, in_maps, core_ids=list(range(NCORES)), trace=trace)
    y = np.asarray(res.results[0]["out"], np.float32).reshape(1)
    return y, res


def kernel(**inputs) -> np.ndarray:
    y, _ = run(**inputs)
    return y
